# revision 2
# baseline (speedup 1.0000x reference)
"""Binary tree-LSTM (BinaryTokenTreeModel) Trainium2 kernel.

Problem: complete binary tree, depth 15 (N=32767 nodes), tree-LSTM with
state size 2H=512, gates 4*2H=2048, vocab 32.  Reference processes nodes
leaves-first; node i's input state is the concat of the first H=256 dims
of its two children's states.

Strategy (8 NeuronCores):
  * Data-parallel over 8 subtrees rooted at the 8 level-3 nodes (7..14).
    Each core runs a level-synchronous scan over its subtree (levels
    13..3 of the global tree), 2047 non-leaf nodes per core.
  * VOCAB=32 => x_proj = (W_ih @ emb.T + b) gathered by type: a 32-column
    table, folded into the level matmul as a one-hot contraction block
    (K = 256+256+32 = 544).
  * Leaf states take only 32 distinct values: precomputed tables
    (host, O(32) work).  Level 13's whole input contraction collapses to
    K=96 of one-hots, and the 16384 leaf output rows are a host-side
    gather of the 32-row table (zero arithmetic).
  * Top 7 nodes (levels 2..0): each core emits its subtree-root (h, c);
    the host finishes the 7-node chain in numpy (15 MFLOP, exact fp32).
    Collectives are avoided entirely (NRT profiling hangs on CC NEFFs).
  * Matmul operands are float16 (full-rate 1 col/cycle PE streaming,
    ~5e-4 rounding on W and h only); accumulation and all elementwise
    math stay fp32.

Self-contained: hardcodes all shapes; only needs numpy + the concourse
(bass) toolchain that ships with the environment.
"""

import sys

for _p in ("/opt/trn_rl_repo", "/root/.axon_site/_ro/trn_rl_repo"):
    if _p not in sys.path:
        sys.path.append(_p)

import numpy as np

import concourse.bacc as bacc
import concourse.mybir as mybir
import concourse.tile as tile
from concourse.bass_utils import run_bass_kernel_spmd

F32 = mybir.dt.float32
F16 = mybir.dt.float16
AF = mybir.ActivationFunctionType

N_CORES = 8
N = 32767
H = 256
H2 = 512
G = 2048  # 4 * H2
V = 32
LEAF0 = (1 << 14) - 1  # 16383: first leaf node id

# Gate column order: critical half (state dims 0:256) then deferred half
# (dims 256:512); within each half [i f o g] so sigmoid spans cols 0:768
# and tanh cols 768:1024 of each 1024-wide half.
GATE_PERM = np.concatenate([
    np.arange(0, 256), np.arange(512, 768),          # i_c f_c
    np.arange(1536, 1792), np.arange(1024, 1280),    # o_c g_c
    np.arange(256, 512), np.arange(768, 1024),       # i_d f_d
    np.arange(1792, 2048), np.arange(1280, 1536),    # o_d g_d
])

# (level, nodes-per-core, output row offset in the per-core out tensor)
PLAN = [
    (13, 1024, 0), (12, 512, 1024), (11, 256, 1536), (10, 128, 1792),
    (9, 64, 1920), (8, 32, 1984), (7, 16, 2016), (6, 8, 2032),
    (5, 4, 2040), (4, 2, 2044), (3, 1, 2046),
]
OUT_ROWS = 2048  # 2047 h rows + 1 root-c row
OHS_OFF = {12: 0, 11: 512, 10: 768, 9: 896, 8: 960, 7: 992, 6: 1008,
           5: 1016, 4: 1020, 3: 1022}

_BUILT = None  # cached (nc, input_names)


def _sigmoid(x):
    return 1.0 / (1.0 + np.exp(-x))


class _Stor:
    """Per-level stationary-input storage (filled by the child level)."""

    def __init__(self, nc, L, M):
        self.M = M
        nch = max(1, (M + 127) // 128)
        mk = lambda n, sh, dt: nc.alloc_sbuf_tensor(f"{n}_{L}", sh, dt).ap()
        self.sA0 = mk("sA0", [128, M], F16)
        self.sA1 = mk("sA1", [128, M], F16)
        self.sB0 = mk("sB0", [128, M], F16)
        self.sB1 = mk("sB1", [128, M], F16)
        self.cin = mk("cin", [min(128, M), nch * 512], F32)


def _build_program(nc):
    din = {}
    for name, shape in [
        ("wk0", [128, G]), ("wk1", [128, G]), ("wk2", [128, G]), ("wk3", [128, G]),
        ("woh", [32, G]), ("w13", [96, G]),
        ("oh3", [96, 1024]), ("ohs", [32, 1023]),
    ]:
        din[name] = nc.dram_tensor(name, shape, F16, kind="ExternalInput").ap()
    din["eye"] = nc.dram_tensor("eye", [128, 128], F32, kind="ExternalInput").ap()
    din["cin13"] = nc.dram_tensor("cin13", [1024, 512], F32, kind="ExternalInput").ap()
    out_d = nc.dram_tensor("out", [OUT_ROWS, 512], F32, kind="ExternalOutput").ap()

    sb = lambda n, sh: nc.alloc_sbuf_tensor(n, sh, F32).ap()
    sbh = lambda n, sh: nc.alloc_sbuf_tensor(n, sh, F16).ap()
    wk = [sbh(f"wk{i}_s", [128, G]) for i in range(4)]
    woh_s = sbh("woh_s", [32, G])
    w13_s = sbh("w13_s", [96, G])
    oh3_s = sbh("oh3_s", [96, 1024])
    ohs_s = sbh("ohs_s", [32, 1023])
    eye_s = sb("eye_s", [128, 128])
    cin13_s = sb("cin13_s", [128, 8 * 512])

    stor = {L: _Stor(nc, L, M) for (L, M, _) in PLAN if L != 13}

    with tile.TileContext(nc) as tc:
        import contextlib

        with contextlib.ExitStack() as ctx:
            gc_pool = ctx.enter_context(
                tc.tile_pool(name="gc", bufs=2, space="PSUM"))
            gd_pool = ctx.enter_context(
                tc.tile_pool(name="gd", bufs=2, space="PSUM"))
            sig_pool = ctx.enter_context(tc.tile_pool(name="sig", bufs=3))
            cell_pool = ctx.enter_context(tc.tile_pool(name="cell", bufs=2))

            # weight / one-hot loads; L13's operands first, halves split
            # across the two HWDGE queues (sync + scalar)
            nc.sync.dma_start(w13_s[0:48], din["w13"][0:48])
            nc.scalar.dma_start(w13_s[48:96], din["w13"][48:96])
            nc.sync.dma_start(oh3_s[0:48], din["oh3"][0:48])
            nc.scalar.dma_start(oh3_s[48:96], din["oh3"][48:96])
            for k in range(8):
                (nc.scalar if k % 2 else nc.sync).dma_start(
                    cin13_s[:, k * 512:(k + 1) * 512],
                    din["cin13"][k * 128:(k + 1) * 128, :])
            for d, s in [
                (din["wk0"], wk[0]), (din["wk2"], wk[2]),
                (din["woh"], woh_s), (din["eye"], eye_s),
            ]:
                nc.sync.dma_start(s, d)
            for d, s in [
                (din["wk1"], wk[1]), (din["wk3"], wk[3]),
                (din["ohs"], ohs_s),
            ]:
                nc.scalar.dma_start(s, d)

            # HAM warm-up: ~12 junk matmuls as soon as w13 lands keep the
            # PE busy through the cold window so L13 runs at 2.4 GHz
            wtile = gc_pool.tile([128, 1024], F32, tag="gc")
            for _ in range(12):
                nc.tensor.matmul(wtile[0:128, 0:512], w13_s[:, 0:128],
                                 w13_s[:, 0:512], start=True, stop=True,
                                 skip_group_check=True)

            def feed_parent(parent, gtile, hsrc, csrc, P, ci):
                """Write child chunk crit states into parent stationary storage.

                Transposes reuse a dead gates PSUM tile of the same chunk
                (banks 0 and 1): the defer tile on fused levels (freed right
                after sig_d/tg_d), the crit tile on split levels.
                hsrc: [P, 256] h crit; csrc: [P, >=256] cols 0:256 c crit."""
                half = P // 2
                base = ci * 64
                t0 = gtile[0:128, 0:P]
                nc.tensor.transpose(t0, hsrc[:, 0:128], eye_s[0:P, 0:P])
                t1 = gtile[0:128, 512:512 + P]
                nc.tensor.transpose(t1, hsrc[:, 128:256], eye_s[0:P, 0:P])
                nc.vector.tensor_copy(parent.sA0[:, base:base + half], t0[:, 0:P:2])
                nc.vector.tensor_copy(parent.sA1[:, base:base + half], t1[:, 0:P:2])
                nc.vector.tensor_copy(parent.sB0[:, base:base + half], t0[:, 1:P:2])
                nc.vector.tensor_copy(parent.sB1[:, base:base + half], t1[:, 1:P:2])
                dr = base % 128
                cb = (ci // 2) * 512
                nc.sync.dma_start(parent.cin[dr:dr + half, cb:cb + 256],
                                  csrc[0:P:2, 0:256])
                nc.sync.dma_start(parent.cin[dr:dr + half, cb + 256:cb + 512],
                                  csrc[1:P:2, 0:256])

            def emit_mms(gtile, lhs_tiles, ws, col0, P):
                """k-outer accumulation of one 1024-col gate half."""
                nk = len(lhs_tiles)
                for k in range(nk):
                    for b in range(2):
                        nc.tensor.matmul(
                            gtile[0:P, b * 512:(b + 1) * 512],
                            lhs_tiles[k],
                            ws[k][:, col0 + b * 512:col0 + (b + 1) * 512],
                            start=(k == 0), stop=(k == nk - 1),
                            skip_group_check=True)

            for (L, M, row_off) in PLAN:
                nch = max(1, (M + 127) // 128)
                fused = M >= 256
                feeds = []
                for pk in range(nch):
                    P = min(128, M - pk * 128)
                    c0 = pk * 128
                    gc = gc_pool.tile([128, 1024], F32)
                    gd = gd_pool.tile([128, 1024], F32)
                    if L == 13:
                        lhs_tiles = [oh3_s[:, c0:c0 + P]]
                        ws = [w13_s]
                        cin_ap = cin13_s[0:P, pk * 512:(pk + 1) * 512]
                    else:
                        st = stor[L]
                        oh_ap = ohs_s[:, OHS_OFF[L] + c0:OHS_OFF[L] + c0 + P]
                        lhs_tiles = [st.sA0[:, c0:c0 + P], st.sA1[:, c0:c0 + P],
                                     st.sB0[:, c0:c0 + P], st.sB1[:, c0:c0 + P],
                                     oh_ap]
                        ws = wk + [woh_s]
                        cin_ap = st.cin[0:P, pk * 512:(pk + 1) * 512]
                    emit_mms(gc, lhs_tiles, ws, 0, P)
                    emit_mms(gd, lhs_tiles, ws, 1024, P)

                    cnew = cell_pool.tile([128, 512], F32)
                    hnew = cell_pool.tile([128, 512], F32)
                    cin3 = cin_ap.rearrange("p (j c) -> p j c", j=2)

                    if fused:
                        sig = sig_pool.tile([128, 1536], F32)
                        nc.scalar.activation(sig[0:P, 0:768], gc[0:P, 0:768],
                                             AF.Sigmoid)
                        nc.scalar.activation(sig[0:P, 768:1536], gd[0:P, 0:768],
                                             AF.Sigmoid)
                        tg = cell_pool.tile([128, 512], F32)
                        nc.scalar.activation(tg[0:P, 0:256], gc[0:P, 768:1024],
                                             AF.Tanh)
                        nc.scalar.activation(tg[0:P, 256:512], gd[0:P, 768:1024],
                                             AF.Tanh)
                        sig3 = sig[0:P].rearrange("p (j c) -> p j c", j=2)
                        tg3 = tg[0:P].rearrange("p (j c) -> p j c", j=2)
                        prod = cell_pool.tile([128, 512], F32)
                        prod3 = prod[0:P].rearrange("p (j c) -> p j c", j=2)
                        nc.vector.tensor_mul(prod3, sig3[:, :, 0:256], tg3)
                        fc = cell_pool.tile([128, 512], F32)
                        fc3 = fc[0:P].rearrange("p (j c) -> p j c", j=2)
                        nc.gpsimd.tensor_mul(fc3, sig3[:, :, 256:512], cin3)
                        nc.vector.tensor_add(cnew[0:P], fc[0:P], prod[0:P])
                        tcc = cell_pool.tile([128, 512], F32)
                        nc.scalar.activation(tcc[0:P], cnew[0:P], AF.Tanh)
                        tcc3 = tcc[0:P].rearrange("p (j c) -> p j c", j=2)
                        hnew3 = hnew[0:P].rearrange("p (j c) -> p j c", j=2)
                        nc.gpsimd.tensor_mul(hnew3, sig3[:, :, 512:768], tcc3)
                    else:
                        # split cell: critical half first (minimum latency),
                        # deferred half after the feed
                        sigc = cell_pool.tile([128, 768], F32, tag="sigc")
                        nc.scalar.activation(sigc[0:P], gc[0:P, 0:768],
                                             AF.Sigmoid)
                        tgc = cell_pool.tile([128, 256], F32, tag="tgc")
                        nc.scalar.activation(tgc[0:P], gc[0:P, 768:1024],
                                             AF.Tanh)
                        prodc = cell_pool.tile([128, 256], F32, tag="prodc")
                        nc.vector.tensor_mul(prodc[0:P], sigc[0:P, 0:256],
                                             tgc[0:P])
                        fcc = cell_pool.tile([128, 256], F32, tag="fcc")
                        nc.vector.tensor_mul(fcc[0:P], sigc[0:P, 256:512],
                                             cin_ap[:, 0:256])
                        nc.vector.tensor_add(cnew[0:P, 0:256], fcc[0:P],
                                             prodc[0:P])
                        tccc = cell_pool.tile([128, 256], F32, tag="tccc")
                        nc.scalar.activation(tccc[0:P], cnew[0:P, 0:256],
                                             AF.Tanh)
                        nc.vector.tensor_mul(hnew[0:P, 0:256],
                                             sigc[0:P, 512:768], tccc[0:P])
                        if L > 3:
                            feed_parent(stor[L - 1], gc, hnew[0:P, 0:256],
                                        cnew[0:P], P, pk)
                        # deferred half (fills engine gaps; GPSIMD-heavy)
                        sigd = cell_pool.tile([128, 768], F32, tag="sigd")
                        nc.scalar.activation(sigd[0:P], gd[0:P, 0:768],
                                             AF.Sigmoid)
                        tgd = cell_pool.tile([128, 256], F32, tag="tgd")
                        nc.scalar.activation(tgd[0:P], gd[0:P, 768:1024],
                                             AF.Tanh)
                        prodd = cell_pool.tile([128, 256], F32, tag="prodd")
                        nc.gpsimd.tensor_mul(prodd[0:P], sigd[0:P, 0:256],
                                             tgd[0:P])
                        fcd = cell_pool.tile([128, 256], F32, tag="fcd")
                        nc.gpsimd.tensor_mul(fcd[0:P], sigd[0:P, 256:512],
                                             cin_ap[:, 256:512])
                        nc.gpsimd.tensor_add(cnew[0:P, 256:512], fcd[0:P],
                                             prodd[0:P])
                        tccd = cell_pool.tile([128, 256], F32, tag="tccd")
                        nc.scalar.activation(tccd[0:P], cnew[0:P, 256:512],
                                             AF.Tanh)
                        nc.gpsimd.tensor_mul(hnew[0:P, 256:512],
                                             sigd[0:P, 512:768], tccd[0:P])

                    nc.sync.dma_start(
                        out_d[row_off + c0:row_off + c0 + P, :], hnew[0:P])

                    if L == 3:
                        nc.sync.dma_start(out_d[2047:2048, :], cnew[0:1])
                    elif fused:
                        feeds.append((gd, hnew, cnew, P, pk))

                for (gd, hnew, cnew, P, pk) in feeds:
                    feed_parent(stor[L - 1], gd, hnew[0:P, 0:256],
                                cnew[0:P], P, pk)

    nc.compile()
    return [k for k in din]


def _get_built():
    global _BUILT
    if _BUILT is None:
        nc = bacc.Bacc("TRN2", target_bir_lowering=False, debug=False,
                       num_devices=N_CORES)
        names = _build_program(nc)
        _BUILT = (nc, names)
    return _BUILT


def kernel(types, a_idx, b_idx, emb, W_ih, W_hh, b_ih, b_hh):
    types = np.asarray(types, np.int32)
    emb = np.asarray(emb, np.float32)
    W_ih = np.asarray(W_ih, np.float32)
    W_hh = np.asarray(W_hh, np.float32)
    b = np.asarray(b_ih, np.float32) + np.asarray(b_hh, np.float32)

    # ---- host weight reparameterization (O(V), no O(N) arithmetic) ----
    XT = (W_ih @ emb.T + b[:, None]).astype(np.float32)          # [2048, 32]
    c_leaf = _sigmoid(XT[0:512]) * np.tanh(XT[1024:1536])        # [512, 32]
    h_leaf = _sigmoid(XT[1536:2048]) * np.tanh(c_leaf)           # [512, 32]
    M_A = W_hh[:, 0:256] @ h_leaf[0:256]                         # [2048, 32]
    M_B = W_hh[:, 256:512] @ h_leaf[0:256]
    w13 = np.ascontiguousarray(
        np.vstack([M_A.T, M_B.T, XT.T])[:, GATE_PERM], np.float16)
    cl256 = np.ascontiguousarray(c_leaf[0:256].T)  # [32, 256]
    W_augT = np.vstack([W_hh.T, XT.T])[:, GATE_PERM]             # [544, 2048]
    wk = [np.ascontiguousarray(W_augT[i * 128:(i + 1) * 128], np.float16)
          for i in range(4)]
    woh = np.ascontiguousarray(W_augT[512:544], np.float16)
    eye = np.eye(128, dtype=np.float32)

    in_maps = []
    for j in range(N_CORES):
        # level 13: one-hots of (left-leaf, right-leaf, self) types
        base13 = (1 << 13) - 1 + j * 1024
        n = np.arange(base13, base13 + 1024)
        oh3 = np.zeros((96, 1024), np.float16)
        m = np.arange(1024)
        oh3[types[2 * n + 1], m] = 1.0
        oh3[32 + types[2 * n + 2], m] = 1.0
        oh3[64 + types[n], m] = 1.0
        cin13 = np.concatenate(
            [cl256[types[2 * n + 1]], cl256[types[2 * n + 2]]], axis=1)
        ohs = np.zeros((32, 1023), np.float16)
        for L in range(12, 2, -1):
            mm = 1 << (L - 3)
            basel = (1 << L) - 1 + j * mm
            off = OHS_OFF[L]
            ohs[types[basel:basel + mm], off + np.arange(mm)] = 1.0
        in_maps.append({
            "wk0": wk[0], "wk1": wk[1], "wk2": wk[2], "wk3": wk[3],
            "woh": woh, "w13": w13, "cin13": cin13,
            "oh3": oh3, "ohs": ohs, "eye": eye,
        })

    nc, _ = _get_built()
    res = run_bass_kernel_spmd(nc, in_maps, core_ids=list(range(N_CORES)))
    global LAST_RESULT
    LAST_RESULT = res

    out = np.empty((N, H2), np.float32)
    for j in range(N_CORES):
        r = res.results[j]["out"]
        off = 0
        for L in range(13, 2, -1):
            mm = 1 << (L - 3)
            basel = (1 << L) - 1 + j * mm
            out[basel:basel + mm] = r[off:off + mm]
            off += mm
    out[LEAF0:] = h_leaf.T[types[LEAF0:]]

    # top 7 nodes (levels 2..0) on host, exactly mirroring the reference
    Hs = np.zeros((15, H2), np.float32)
    Cs = np.zeros((15, H2), np.float32)
    for j in range(N_CORES):
        Hs[7 + j] = res.results[j]["out"][2046]
        Cs[7 + j] = res.results[j]["out"][2047]
    for n in range(6, -1, -1):
        a, bb = 2 * n + 1, 2 * n + 2
        hin = np.concatenate([Hs[a, :H], Hs[bb, :H]])
        cin = np.concatenate([Cs[a, :H], Cs[bb, :H]])
        gates = XT[:, types[n]] + W_hh @ hin
        ig, fg, gg, og = np.split(gates, 4)
        c_new = _sigmoid(fg) * cin + _sigmoid(ig) * np.tanh(gg)
        h_new = _sigmoid(og) * np.tanh(c_new)
        Hs[n] = h_new
        Cs[n] = c_new
        out[n] = h_new
    return out



# revision 9
# speedup vs baseline: 1.2119x; 1.2119x over previous
"""Binary tree-LSTM (BinaryTokenTreeModel) Trainium2 kernel, v2.

Problem: complete binary tree, depth 15 (N=32767), tree-LSTM with state
2H=512, gates 4*2H=2048, vocab 32.  Children feed parents the first
H=256 dims of (h, c).

Design (8 NeuronCores, data-parallel over the 8 level-3 subtrees):
  * Device computes levels 13..8 of each subtree (2016 nodes/core);
    leaves are a host-side 32-entry table; levels 7..0 (255 nodes) are
    finished on host with level-batched GEMMs.
  * Unzip storage order per level (left children first, then right):
    feeds to the parent's stationary matmul tiles become whole
    [128,128] transposes and contiguous column splits -- no strided
    gathers; incoming c is read directly from the child's tile.
  * sigmoid-only activations: g-gate weight rows are pre-scaled x2 so
    tanh(g) = 2*sigmoid(2g)-1 and tanh(c) = 2*sigmoid(2c)-1; each chunk
    needs only 3 ACT calls (the scalar engine is the throughput floor).
  * One-hot type contraction (K=32) row-replicated 4x so the four
    quadrant matmuls run concurrently in distinct PE row groups.
  * Software-pipelined emission (gates matmuls run two chunks ahead of
    the cell math) keeps the PE dense so the HAM clock gate stays warm;
    junk matmuls cover the initial DMA window.
  * Matmul operands fp16 (weights, child h, one-hots); everything else
    fp32.

Self-contained: hardcodes all shapes; needs only numpy + the concourse
(bass) toolchain shipped with the environment.
"""

import sys

for _p in ("/opt/trn_rl_repo", "/root/.axon_site/_ro/trn_rl_repo"):
    if _p not in sys.path:
        sys.path.append(_p)

import numpy as np

import concourse.bacc as bacc
import concourse.mybir as mybir
import concourse.tile as tile
from concourse.alu_op_type import AluOpType
from concourse.bass_utils import run_bass_kernel_spmd

F32 = mybir.dt.float32
F16 = mybir.dt.float16
AF = mybir.ActivationFunctionType

N_CORES = 8
N = 32767
H = 256
H2 = 512
G = 2048
V = 32
LEAF0 = (1 << 14) - 1

# Permuted gate layout: [i_c f_c o_c g_c | i_d f_d o_d g_d] (c = state dims
# 0:256 fed to the parent, d = dims 256:512).  Torch row order is i,f,g,o.
GATE_PERM = np.concatenate([
    np.arange(0, 256), np.arange(512, 768),
    np.arange(1536, 1792), np.arange(1024, 1280),
    np.arange(256, 512), np.arange(768, 1024),
    np.arange(1792, 2048), np.arange(1280, 1536),
])
GCOLS = np.concatenate([np.arange(768, 1024), np.arange(1792, 2048)])

# (level, nodes-per-core, out row offset)
PLAN = [(13, 1024, 0), (12, 512, 1024), (11, 256, 1536), (10, 128, 1792),
        (9, 64, 1920), (8, 32, 1984)]
OHS_OFF = {12: 0, 11: 512, 10: 768, 9: 896, 8: 960}
OHS_W = 992
OUT_ROWS = 2048  # 2016 h rows + 32 level-8 c rows

# big16 column offsets
WK_OFF = 0            # wk0..wk3, 2048 each
WOH_OFF = 8192        # wohrep [128, 2048]
W13_OFF = 10240       # w13 rows 0:96 [*, 2048]
OH3_OFF = 12288       # oh3 rows 0:96 [*, 1024]
OHS_COFF = 13312      # ohsrep [128, 992]
C16 = 14304
# big32 column offsets
EYE_OFF = 0
CIN13_OFF = 128
C32 = 128 + 4096

_BUILT = None


def _sigmoid(x):
    return 1.0 / (1.0 + np.exp(-x))


def _perms():
    sig = {8: np.arange(32)}
    for L in range(9, 14):
        p = sig[L - 1]
        sig[L] = np.concatenate([2 * p, 2 * p + 1])
    return sig


class _Stor:
    def __init__(self, nc, L, M):
        self.M = M
        mk = lambda n: nc.alloc_sbuf_tensor(f"{n}_{L}", [128, M], F16).ap()
        self.sA0 = mk("sA0")
        self.sA1 = mk("sA1")
        self.sB0 = mk("sB0")
        self.sB1 = mk("sB1")


def _build_program(nc):
    big16 = nc.dram_tensor("big16", [128, C16], F16, kind="ExternalInput").ap()
    big32 = nc.dram_tensor("big32", [128, C32], F32, kind="ExternalInput").ap()
    out_d = nc.dram_tensor("out", [OUT_ROWS, 512], F32, kind="ExternalOutput").ap()

    b16 = nc.alloc_sbuf_tensor("b16s", [128, C16], F16).ap()
    b32 = nc.alloc_sbuf_tensor("b32s", [128, C32], F32).ap()
    wk = [b16[:, WK_OFF + 2048 * k:WK_OFF + 2048 * (k + 1)] for k in range(4)]
    wohrep = b16[:, WOH_OFF:WOH_OFF + 2048]
    w13 = b16[0:96, W13_OFF:W13_OFF + 2048]
    oh3 = b16[0:96, OH3_OFF:OH3_OFF + 1024]
    ohsrep = b16[:, OHS_COFF:OHS_COFF + OHS_W]
    eye = b32[:, EYE_OFF:EYE_OFF + 128]
    cin13 = b32[:, CIN13_OFF:CIN13_OFF + 4096]

    # per-level c storage (fp32) and parent stationaries (fp16)
    c_lev = {L: nc.alloc_sbuf_tensor(f"c{L}", [128, (max(M, 128) // 128) * 512],
                                     F32).ap()
             for (L, M, _) in PLAN}
    stor = {L: _Stor(nc, L, M) for (L, M, _) in PLAN if L != 13}
    cB9 = nc.alloc_sbuf_tensor("cB9", [64, 256], F32).ap()
    cB8 = nc.alloc_sbuf_tensor("cB8", [32, 256], F32).ap()

    with tile.TileContext(nc) as tc:
        import contextlib

        with contextlib.ExitStack() as ctx:
            gc_pool = ctx.enter_context(
                tc.tile_pool(name="gc", bufs=2, space="PSUM"))
            gd_pool = ctx.enter_context(
                tc.tile_pool(name="gd", bufs=2, space="PSUM"))
            sig_pool = ctx.enter_context(tc.tile_pool(name="sig", bufs=3))
            s2c_pool = ctx.enter_context(tc.tile_pool(name="s2c", bufs=3))
            work_pool = ctx.enter_context(tc.tile_pool(name="wrk", bufs=8))
            h_pool = ctx.enter_context(tc.tile_pool(name="hh", bufs=3))

            # ---- input DMA, ordered by first use, split over 2 queues ----
            nc.sync.dma_start(b32[:, 0:128], big32[:, 0:128])          # eye
            nc.scalar.dma_start(b16[:, W13_OFF:W13_OFF + 3072],
                                big16[:, W13_OFF:W13_OFF + 3072])      # w13+oh3
            nc.sync.dma_start(b32[:, 128:4224], big32[:, 128:4224])    # cin13
            nc.scalar.dma_start(b16[:, OHS_COFF:OHS_COFF + OHS_W],
                                big16[:, OHS_COFF:OHS_COFF + OHS_W])   # ohsrep
            nc.sync.dma_start(b16[:, 0:4096], big16[:, 0:4096])        # wk01
            nc.scalar.dma_start(b16[:, 4096:8192], big16[:, 4096:8192])  # wk23
            nc.sync.dma_start(b16[:, 8192:10240], big16[:, 8192:10240])  # wohrep

            # ---- HAM warm-up: junk matmuls bridge the input-DMA window ----
            wtile = gc_pool.tile([128, 1024], F32, tag="gc")
            for _ in range(24):
                nc.tensor.matmul(wtile[0:128, 0:512],
                                 b16[:, W13_OFF:W13_OFF + 128],
                                 b16[:, W13_OFF:W13_OFF + 512],
                                 start=True, stop=True, skip_group_check=True)

            def emit_A(L, pk, P, gc, gd):
                """Gates matmuls for one chunk into gc (crit) / gd (defer)."""
                c0 = pk * 128
                quads = [gc[0:P, 0:512], gc[0:P, 512:1024],
                         gd[0:P, 0:512], gd[0:P, 512:1024]]
                if L == 13:
                    lhs = oh3[:, c0:c0 + P]
                    for b in range(4):
                        nc.tensor.matmul(quads[b], lhs,
                                         w13[:, 512 * b:512 * (b + 1)],
                                         start=True, stop=True,
                                         skip_group_check=True)
                else:
                    st = stor[L]
                    lhs4 = [st.sA0[:, c0:c0 + P], st.sA1[:, c0:c0 + P],
                            st.sB0[:, c0:c0 + P], st.sB1[:, c0:c0 + P]]
                    for b in range(4):
                        for k in range(4):
                            nc.tensor.matmul(
                                quads[b], lhs4[k],
                                wk[k][:, 512 * b:512 * (b + 1)],
                                start=(k == 0), stop=False,
                                skip_group_check=True)
                    off = OHS_OFF[L]
                    for b in range(4):
                        nc.tensor.matmul(
                            quads[b],
                            ohsrep[32 * b:32 * b + 32, off + c0:off + c0 + P],
                            wohrep[32 * b:32 * b + 32, 512 * b:512 * (b + 1)],
                            start=False, stop=True, skip_group_check=True,
                            tile_position=(32 * b, 0))

            def sigc(P, gc, sg):
                nc.scalar.activation(sg[0:P, 0:1024], gc[0:P, 0:1024],
                                     AF.Sigmoid)

            def sigd(P, gd, sg):
                nc.scalar.activation(sg[0:P, 1024:2048], gd[0:P, 0:1024],
                                     AF.Sigmoid)

            def cell_half(P, sg, coff, cin_ap, cdst, hdst):
                """One 256-dim half of the cell from sigmoid tile sg.

                tanh(x) = 2σ(2x)−1 (g weights pre-scaled ×2):
                  c = σf·c_in + σi·(2σg−1);  h = σo·(2σ(2c)−1)
                coff: 0 (crit) or 1024 (defer).  cdst/hdst: [P, 256]."""
                si = sg[0:P, coff + 0:coff + 256]
                sf = sg[0:P, coff + 256:coff + 512]
                so = sg[0:P, coff + 512:coff + 768]
                sgg = sg[0:P, coff + 768:coff + 1024]
                tg = work_pool.tile([128, 256], F32)
                nc.vector.tensor_scalar(tg[0:P], sgg, 2.0, -1.0,
                                        AluOpType.mult, AluOpType.add)
                p2 = work_pool.tile([128, 256], F32)
                nc.vector.tensor_mul(p2[0:P], si, tg[0:P])
                fc = work_pool.tile([128, 256], F32)
                nc.gpsimd.tensor_mul(fc[0:P], sf, cin_ap)
                nc.gpsimd.tensor_add(cdst, fc[0:P], p2[0:P])
                s2c = s2c_pool.tile([128, 256], F32)
                nc.scalar.activation(s2c[0:P], cdst, AF.Sigmoid, scale=2.0)
                tc = work_pool.tile([128, 256], F32)
                nc.vector.tensor_scalar(tc[0:P], s2c[0:P], 2.0, -1.0,
                                        AluOpType.mult, AluOpType.add)
                nc.gpsimd.tensor_mul(hdst, so, tc[0:P])

            def cell_fused(P, sg, cA, cB, cdst512, hnew):
                """Full-width cell using crit/defer paired (j=2) APs."""
                sg3 = sg[0:P].rearrange("p (j c) -> p j c", j=2)
                tg = work_pool.tile([128, 512], F32)
                tg3 = tg[0:P].rearrange("p (j c) -> p j c", j=2)
                nc.vector.tensor_scalar(tg3, sg3[:, :, 768:1024], 2.0, -1.0,
                                        AluOpType.mult, AluOpType.add)
                p2 = work_pool.tile([128, 512], F32)
                p23 = p2[0:P].rearrange("p (j c) -> p j c", j=2)
                nc.vector.tensor_mul(p23, sg3[:, :, 0:256], tg3)
                fc = work_pool.tile([128, 512], F32)
                nc.gpsimd.tensor_mul(fc[0:P, 0:256], sg[0:P, 256:512], cA)
                nc.gpsimd.tensor_mul(fc[0:P, 256:512], sg[0:P, 1280:1536], cB)
                nc.gpsimd.tensor_add(cdst512, fc[0:P], p2[0:P])
                s2c = s2c_pool.tile([128, 512], F32)
                nc.scalar.activation(s2c[0:P], cdst512, AF.Sigmoid, scale=2.0)
                tc = work_pool.tile([128, 512], F32)
                nc.vector.tensor_scalar(tc[0:P], s2c[0:P], 2.0, -1.0,
                                        AluOpType.mult, AluOpType.add)
                h3 = hnew[0:P].rearrange("p (j c) -> p j c", j=2)
                nc.gpsimd.tensor_mul(h3, sg3[:, :, 512:768],
                                     tc[0:P].rearrange("p (j c) -> p j c", j=2))

            def feed(L, pk, nch, P, hnew, gtile):
                """Transpose h crit into the parent's stationary tiles.

                Uses the dead gc PSUM tile of the same chunk as transpose
                scratch (banks 0 and 1)."""
                par = stor[L - 1]
                t0 = gtile[0:128, 0:P]
                t1 = gtile[0:128, 512:512 + P]
                nc.tensor.transpose(t0, hnew[0:P, 0:128], eye[0:P, 0:P])
                nc.tensor.transpose(t1, hnew[0:P, 128:256], eye[0:P, 0:P])
                if nch >= 2:
                    half = nch // 2
                    if pk < half:
                        d0, d1, col = par.sA0, par.sA1, 128 * pk
                    else:
                        d0, d1, col = par.sB0, par.sB1, 128 * (pk - half)
                    nc.vector.tensor_copy(d0[:, col:col + 128], t0)
                    nc.vector.tensor_copy(d1[:, col:col + 128], t1)
                else:
                    mp = P // 2
                    nc.vector.tensor_copy(par.sA0[:, 0:mp], t0[:, 0:mp])
                    nc.vector.tensor_copy(par.sB0[:, 0:mp], t0[:, mp:P])
                    nc.vector.tensor_copy(par.sA1[:, 0:mp], t1[:, 0:mp])
                    nc.vector.tensor_copy(par.sB1[:, 0:mp], t1[:, mp:P])

            def cin_aps(L, pk, P):
                """(cA, cB) incoming-c APs for chunk pk of level L."""
                if L == 13:
                    base = 128 + 512 * pk
                    return (b32[0:P, base:base + 256],
                            b32[0:P, base + 256:base + 512])
                child = L + 1
                nch_c = max(1, (2 * {l: m for (l, m, _) in PLAN}[L]) // 128)
                cc = c_lev[child]
                if nch_c >= 2:
                    half = nch_c // 2
                    return (cc[0:P, 512 * pk:512 * pk + 256],
                            cc[0:P, 512 * (half + pk):512 * (half + pk) + 256])
                if L == 9:
                    return (c_lev[10][0:64, 0:256], cB9[0:64, 0:256])
                return (c_lev[9][0:32, 0:256], cB8[0:32, 0:256])

            # ---------- fused levels 13..11 (software-pipelined) ----------
            for (L, M, row_off) in PLAN[:3]:
                nch = M // 128
                pend = []

                def emit_C(st):
                    (Lc, pkc, nchc, Pc, gcc, gdc, sgc, roff) = st
                    cA, cB = cin_aps(Lc, pkc, Pc)
                    hnew = h_pool.tile([128, 512], F32)
                    cdst = c_lev[Lc][0:Pc, 512 * pkc:512 * pkc + 512]
                    cell_fused(Pc, sgc, cA, cB, cdst, hnew)
                    nc.sync.dma_start(
                        out_d[roff + 128 * pkc:roff + 128 * pkc + Pc, :],
                        hnew[0:Pc])
                    feed(Lc, pkc, nchc, Pc, hnew, gcc)

                for pk in range(nch):
                    if len(pend) == 2:
                        emit_C(pend.pop(0))
                    gc = gc_pool.tile([128, 1024], F32, tag="gc")
                    gd = gd_pool.tile([128, 1024], F32, tag="gd")
                    emit_A(L, pk, 128, gc, gd)
                    sg = sig_pool.tile([128, 2048], F32)
                    sigc(128, gc, sg)
                    sigd(128, gd, sg)
                    pend.append((L, pk, nch, 128, gc, gd, sg, row_off))
                while pend:
                    emit_C(pend.pop(0))

            # ---------- split levels 10, 9 and final level 8 ----------
            for (L, M, row_off) in PLAN[3:]:
                P = M
                gc = gc_pool.tile([128, 1024], F32, tag="gc")
                gd = gd_pool.tile([128, 1024], F32, tag="gd")
                emit_A(L, 0, P, gc, gd)
                sg = sig_pool.tile([128, 2048], F32)
                sigc(P, gc, sg)
                sigd(P, gd, sg)
                cA, cB = cin_aps(L, 0, P)
                hnew = h_pool.tile([128, 512], F32)
                cdst = c_lev[L]
                # crit half first: unblocks the next level's matmuls
                cell_half(P, sg, 0, cA, cdst[0:P, 0:256], hnew[0:P, 0:256])
                if L > 8:
                    feed(L, 0, 1, P, hnew, gc)
                    # stage the B-children c rows for the next level
                    if L == 10:
                        nc.sync.dma_start(cB9[0:64, 0:256],
                                          cdst[64:128, 0:256])
                    else:
                        nc.sync.dma_start(cB8[0:32, 0:256],
                                          cdst[32:64, 0:256])
                cell_half(P, sg, 1024, cB, cdst[0:P, 256:512],
                          hnew[0:P, 256:512])
                nc.sync.dma_start(out_d[row_off:row_off + P, :], hnew[0:P])
                if L == 8:
                    nc.sync.dma_start(out_d[2016:2048, :], cdst[0:32, 0:512])

    nc.compile()


def _get_built():
    global _BUILT
    if _BUILT is None:
        nc = bacc.Bacc("TRN2", target_bir_lowering=False, debug=False,
                       num_devices=N_CORES)
        _build_program(nc)
        _BUILT = nc
    return _BUILT


def kernel(types, a_idx, b_idx, emb, W_ih, W_hh, b_ih, b_hh):
    types = np.asarray(types, np.int32)
    emb = np.asarray(emb, np.float32)
    W_ih = np.asarray(W_ih, np.float32)
    W_hh = np.asarray(W_hh, np.float32)
    b = np.asarray(b_ih, np.float32) + np.asarray(b_hh, np.float32)

    # ---- host weight reparameterization (O(V) work) ----
    XT = (W_ih @ emb.T + b[:, None]).astype(np.float32)      # [2048, 32]
    c_leaf = _sigmoid(XT[0:512]) * np.tanh(XT[1024:1536])
    h_leaf = _sigmoid(XT[1536:2048]) * np.tanh(c_leaf)
    M_A = W_hh[:, 0:256] @ h_leaf[0:256]
    M_B = W_hh[:, 256:512] @ h_leaf[0:256]
    cl256 = np.ascontiguousarray(c_leaf[0:256].T)            # [32, 256]

    w13p = np.vstack([M_A.T, M_B.T, XT.T])[:, GATE_PERM].copy()
    w13p[:, GCOLS] *= 2.0
    w13p = w13p.astype(np.float16)
    W_augT = np.vstack([W_hh.T, XT.T])[:, GATE_PERM].copy()
    W_augT[:, GCOLS] *= 2.0
    W_augT16 = W_augT.astype(np.float16)

    base16 = np.zeros((128, C16), np.float16)
    for k in range(4):
        base16[:, 2048 * k:2048 * (k + 1)] = W_augT16[128 * k:128 * (k + 1)]
    base16[:, WOH_OFF:WOH_OFF + 2048] = np.vstack([W_augT16[512:544]] * 4)
    base16[0:96, W13_OFF:W13_OFF + 2048] = w13p

    eye = np.eye(128, dtype=np.float32)
    sig = _perms()

    in_maps = []
    for j in range(N_CORES):
        n13 = (1 << 13) - 1 + 1024 * j + sig[13]
        la, lb = 2 * n13 + 1, 2 * n13 + 2
        oh3 = np.zeros((96, 1024), np.float16)
        m = np.arange(1024)
        oh3[types[la], m] = 1.0
        oh3[32 + types[lb], m] = 1.0
        oh3[64 + types[n13], m] = 1.0
        cin13 = np.concatenate([cl256[types[la]], cl256[types[lb]]],
                               axis=1).astype(np.float32)    # [1024, 512]
        cimg = cin13.reshape(8, 128, 512).transpose(1, 0, 2).reshape(128, 4096)
        ohs = np.zeros((32, OHS_W), np.float16)
        for (L, M, _) in PLAN[1:]:
            nodes = (1 << L) - 1 + M * j + sig[L]
            ohs[types[nodes], OHS_OFF[L] + np.arange(M)] = 1.0

        b16 = base16.copy()
        b16[0:96, OH3_OFF:OH3_OFF + 1024] = oh3
        b16[:, OHS_COFF:OHS_COFF + OHS_W] = np.vstack([ohs] * 4)
        b32 = np.zeros((128, C32), np.float32)
        b32[:, 0:128] = eye
        b32[:, 128:4224] = cimg
        in_maps.append({"big16": b16, "big32": b32})

    nc = _get_built()
    res = run_bass_kernel_spmd(nc, in_maps, core_ids=list(range(N_CORES)))
    global LAST_RESULT
    LAST_RESULT = res

    out = np.empty((N, H2), np.float32)
    out[LEAF0:] = h_leaf.T[types[LEAF0:]]
    Hn = np.zeros((511, H2), np.float32)
    Cn = np.zeros((511, H2), np.float32)
    for j in range(N_CORES):
        r = res.results[j]["out"]
        for (L, M, off) in PLAN:
            base = (1 << L) - 1 + M * j
            out[base + sig[L]] = r[off:off + M]
        Hn[255 + 32 * j:255 + 32 * (j + 1)] = r[1984:2016]
        Cn[255 + 32 * j:255 + 32 * (j + 1)] = r[2016:2048]

    # host finisher: levels 7..0, level-batched (torch gate order)
    for L in range(7, -1, -1):
        ids = np.arange((1 << L) - 1, (1 << (L + 1)) - 1)
        a, bb = 2 * ids + 1, 2 * ids + 2
        hin = np.concatenate([Hn[a][:, 0:256], Hn[bb][:, 0:256]], axis=1)
        cin = np.concatenate([Cn[a][:, 0:256], Cn[bb][:, 0:256]], axis=1)
        gates = XT[:, types[ids]].T + hin @ W_hh.T
        ig, fg, gg, og = np.split(gates, 4, axis=1)
        c_new = _sigmoid(fg) * cin + _sigmoid(ig) * np.tanh(gg)
        h_new = _sigmoid(og) * np.tanh(c_new)
        Hn[ids] = h_new
        Cn[ids] = c_new
        out[ids] = h_new
    return out


# revision 15
# speedup vs baseline: 1.5657x; 1.2920x over previous
"""Binary tree-LSTM (BinaryTokenTreeModel) Trainium2 kernel, v2b.

Complete binary tree, depth 15 (N=32767), tree-LSTM state 2H=512,
gates 4*2H=2048, vocab 32.  Children feed parents the first H=256 dims
of (h, c).

Design (8 NeuronCores, data-parallel over the 8 level-3 subtrees):
  * Device computes levels 13..8 (2016 nodes/core); leaves are a host
    32-entry table; levels 7..0 (255 nodes) finish on host with
    level-batched GEMMs.
  * Unzip storage order (left children first): parent feeds are whole
    [128,128] transposes and contiguous column splits.
  * sigmoid-only activations (g-gate weight rows pre-scaled x2 so
    tanh(x) = 2*sigmoid(2x)-1): one 2048-wide ACT call per chunk plus
    one 512-wide for tanh(c) -- the scalar engine is the floor.
  * Gates accumulate in a single [128,2048] PSUM tile (4 banks, 2
    bufs); one-hot type rows (K=32) replicated 4x so the four quadrant
    matmuls run concurrently in distinct PE row groups.
  * All cell math in fp16 on the vector engine (2x DVE rate); c state,
    sigma tiles, and the output tensor are fp16 (rel err ~4e-3, budget
    2e-2).
  * Feeds: fused levels use HWDGE transpose-DMAs (no PE, no PSUM);
    split tail levels use PE transposes into the dead gates tile.
  * Software-pipelined emission (matmuls run two chunks ahead of the
    cell chain) keeps the PE dense so the HAM clock gate stays warm;
    junk matmuls cover the initial DMA window.

Self-contained: hardcodes all shapes; needs only numpy + the concourse
(bass) toolchain shipped with the environment.
"""

import sys

for _p in ("/opt/trn_rl_repo", "/root/.axon_site/_ro/trn_rl_repo"):
    if _p not in sys.path:
        sys.path.append(_p)

import numpy as np

import concourse.bacc as bacc
import concourse.mybir as mybir
import concourse.tile as tile
from concourse.alu_op_type import AluOpType
from concourse.bass_utils import run_bass_kernel_spmd

F32 = mybir.dt.float32
F16 = mybir.dt.float16
AF = mybir.ActivationFunctionType

N_CORES = 8
N = 32767
H = 256
H2 = 512
G = 2048
V = 32
LEAF0 = (1 << 14) - 1

# Permuted gate layout: [i_c f_c o_c g_c | i_d f_d o_d g_d]
GATE_PERM = np.concatenate([
    np.arange(0, 256), np.arange(512, 768),
    np.arange(1536, 1792), np.arange(1024, 1280),
    np.arange(256, 512), np.arange(768, 1024),
    np.arange(1792, 2048), np.arange(1280, 1536),
])
GCOLS = np.concatenate([np.arange(768, 1024), np.arange(1792, 2048)])

PLAN = [(13, 1024, 0), (12, 512, 1024), (11, 256, 1536), (10, 128, 1792),
        (9, 64, 1920), (8, 32, 1984)]
OHS_OFF = {12: 0, 11: 512, 10: 768, 9: 896, 8: 960}
OHS_W = 992
OUT_ROWS = 2048  # 2016 h rows + 32 level-8 c rows

# big16 column offsets (single fp16 input tensor)
WK_OFF = 0
WOH_OFF = 8192
W13_OFF = 10240
OH3_OFF = 12288
OHS_COFF = 13312
CIN13_OFF = 14304
EYE_OFF = 18400
C16 = 18528

_BUILT = None


def _sigmoid(x):
    return 1.0 / (1.0 + np.exp(-x))


def _perms():
    sig = {8: np.arange(32)}
    for L in range(9, 14):
        p = sig[L - 1]
        sig[L] = np.concatenate([2 * p, 2 * p + 1])
    return sig


class _Stor:
    def __init__(self, nc, L, M):
        self.M = M
        mk = lambda n: nc.alloc_sbuf_tensor(f"{n}_{L}", [128, M], F16).ap()
        self.sA0 = mk("sA0")
        self.sA1 = mk("sA1")
        self.sB0 = mk("sB0")
        self.sB1 = mk("sB1")


def _build_program(nc):
    big16 = nc.dram_tensor("big16", [128, C16], F16, kind="ExternalInput").ap()
    out_d = nc.dram_tensor("out", [OUT_ROWS, 512], F16, kind="ExternalOutput").ap()

    b16 = nc.alloc_sbuf_tensor("b16s", [128, C16], F16).ap()
    wk = [b16[:, WK_OFF + 2048 * k:WK_OFF + 2048 * (k + 1)] for k in range(4)]
    wohrep = b16[:, WOH_OFF:WOH_OFF + 2048]
    w13 = b16[0:96, W13_OFF:W13_OFF + 2048]
    oh3 = b16[0:96, OH3_OFF:OH3_OFF + 1024]
    ohsrep = b16[:, OHS_COFF:OHS_COFF + OHS_W]
    cin13 = b16[:, CIN13_OFF:CIN13_OFF + 4096]
    eye = b16[:, EYE_OFF:EYE_OFF + 128]

    c_lev = {L: nc.alloc_sbuf_tensor(f"c{L}", [128, (max(M, 128) // 128) * 512],
                                     F16).ap()
             for (L, M, _) in PLAN}
    stor = {L: _Stor(nc, L, M) for (L, M, _) in PLAN if L != 13}
    cB9 = nc.alloc_sbuf_tensor("cB9", [64, 256], F16).ap()
    cB8 = nc.alloc_sbuf_tensor("cB8", [32, 256], F16).ap()

    with tile.TileContext(nc) as tc:
        import contextlib

        with contextlib.ExitStack() as ctx:
            gc_pool = ctx.enter_context(
                tc.tile_pool(name="gc", bufs=2, space="PSUM"))
            gd_pool = ctx.enter_context(
                tc.tile_pool(name="gd", bufs=1, space="PSUM"))
            tp_pool = ctx.enter_context(
                tc.tile_pool(name="tp", bufs=2, space="PSUM"))
            sig_pool = ctx.enter_context(tc.tile_pool(name="sig", bufs=3))
            s2c_pool = ctx.enter_context(tc.tile_pool(name="s2c", bufs=3))
            work_pool = ctx.enter_context(tc.tile_pool(name="wrk", bufs=12))
            h_pool = ctx.enter_context(tc.tile_pool(name="hh", bufs=4))

            # ---- input DMA, ordered by first use, 2 HWDGE queues ----
            nc.scalar.dma_start(b16[:, W13_OFF:W13_OFF + 3072],
                                big16[:, W13_OFF:W13_OFF + 3072])  # w13+oh3
            nc.sync.dma_start(b16[:, CIN13_OFF:C16],
                              big16[:, CIN13_OFF:C16])             # cin13+eye
            nc.scalar.dma_start(b16[:, OHS_COFF:OHS_COFF + OHS_W],
                                big16[:, OHS_COFF:OHS_COFF + OHS_W])
            nc.sync.dma_start(b16[:, 0:4096], big16[:, 0:4096])    # wk01
            nc.scalar.dma_start(b16[:, 4096:8192], big16[:, 4096:8192])
            nc.sync.dma_start(b16[:, 8192:10240], big16[:, 8192:10240])

            # ---- HAM warm-up ----
            wtile = gc_pool.tile([128, 1024], F32, tag="gc")
            for _ in range(24):
                nc.tensor.matmul(wtile[0:128, 0:512],
                                 b16[:, W13_OFF:W13_OFF + 128],
                                 b16[:, W13_OFF:W13_OFF + 512],
                                 start=True, stop=True, skip_group_check=True)

            def emit_A(L, pk, P, gc, gd):
                c0 = pk * 128
                quads = [gc[0:P, 0:512], gc[0:P, 512:1024],
                         gd[0:P, 0:512], gd[0:P, 512:1024]]
                if L == 13:
                    lhs = oh3[:, c0:c0 + P]
                    for b in range(4):
                        nc.tensor.matmul(quads[b], lhs,
                                         w13[:, 512 * b:512 * (b + 1)],
                                         start=True, stop=True,
                                         skip_group_check=True)
                else:
                    st = stor[L]
                    lhs4 = [st.sA0[:, c0:c0 + P], st.sA1[:, c0:c0 + P],
                            st.sB0[:, c0:c0 + P], st.sB1[:, c0:c0 + P]]
                    for b in range(4):
                        for k in range(4):
                            nc.tensor.matmul(
                                quads[b], lhs4[k],
                                wk[k][:, 512 * b:512 * (b + 1)],
                                start=(k == 0), stop=False,
                                skip_group_check=True)
                    off = OHS_OFF[L]
                    for b in range(4):
                        nc.tensor.matmul(
                            quads[b],
                            ohsrep[32 * b:32 * b + 32, off + c0:off + c0 + P],
                            wohrep[32 * b:32 * b + 32, 512 * b:512 * (b + 1)],
                            start=False, stop=True, skip_group_check=True,
                            tile_position=(32 * b, 0))

            def cell_half(P, sg, coff, cin_ap, cdst, hdst, geng=True):
                """One 256-dim half: c = σf·c_in + σi·(2σg−1);
                h = σo·(2σ(2c)−1).  All fp16, 2D contiguous APs."""
                si = sg[0:P, coff + 0:coff + 256]
                sf = sg[0:P, coff + 256:coff + 512]
                so = sg[0:P, coff + 512:coff + 768]
                sgg = sg[0:P, coff + 768:coff + 1024]
                tg = work_pool.tile([128, 256], F16)
                nc.vector.tensor_scalar(tg[0:P], sgg, 2.0, -1.0,
                                        AluOpType.mult, AluOpType.add)
                p2 = work_pool.tile([128, 256], F16)
                nc.vector.tensor_mul(p2[0:P], si, tg[0:P])
                fc = work_pool.tile([128, 256], F16)
                (nc.gpsimd if geng else nc.vector).tensor_mul(
                    fc[0:P], sf, cin_ap)
                nc.vector.tensor_add(cdst, fc[0:P], p2[0:P])
                s2c = s2c_pool.tile([128, 256], F16)
                nc.scalar.activation(s2c[0:P], cdst, AF.Sigmoid, scale=2.0)
                tc_ = work_pool.tile([128, 256], F16)
                nc.vector.tensor_scalar(tc_[0:P], s2c[0:P], 2.0, -1.0,
                                        AluOpType.mult, AluOpType.add)
                (nc.gpsimd if geng else nc.vector).tensor_mul(
                    hdst, so, tc_[0:P])

            def cell_fused(P, sg, cA, cB, cdst, hnew):
                """Both halves; fc on gpsimd, single [P,512] s2c call."""
                for half, coff in ((0, 0), (1, 1024)):
                    si = sg[0:P, coff + 0:coff + 256]
                    sf = sg[0:P, coff + 256:coff + 512]
                    sgg = sg[0:P, coff + 768:coff + 1024]
                    cin_ap = cA if half == 0 else cB
                    cs = slice(256 * half, 256 * half + 256)
                    tg = work_pool.tile([128, 256], F16)
                    nc.vector.tensor_scalar(tg[0:P], sgg, 2.0, -1.0,
                                            AluOpType.mult, AluOpType.add)
                    p2 = work_pool.tile([128, 256], F16)
                    nc.vector.tensor_mul(p2[0:P], si, tg[0:P])
                    fc = work_pool.tile([128, 256], F16)
                    nc.gpsimd.tensor_mul(fc[0:P], sf, cin_ap)
                    nc.vector.tensor_add(cdst[0:P, cs], fc[0:P], p2[0:P])
                s2c = s2c_pool.tile([128, 512], F16)
                nc.scalar.activation(s2c[0:P], cdst[0:P, 0:512],
                                     AF.Sigmoid, scale=2.0)
                for half, coff in ((0, 0), (1, 1024)):
                    so = sg[0:P, coff + 512:coff + 768]
                    cs = slice(256 * half, 256 * half + 256)
                    tc_ = work_pool.tile([128, 256], F16)
                    nc.vector.tensor_scalar(tc_[0:P], s2c[0:P, cs], 2.0, -1.0,
                                            AluOpType.mult, AluOpType.add)
                    (nc.gpsimd if half else nc.vector).tensor_mul(
                        hnew[0:P, cs], sg[0:P, coff + 512:coff + 768],
                        tc_[0:P])

            def feed_pe(L, pk, nch, P, hnew):
                """Feed: fp16 PE transposes of h crit into the transpose
                scratch, then column copies into the parent stationaries."""
                par = stor[L - 1]
                tp = tp_pool.tile([128, 256], F16)
                t0 = tp[0:128, 0:P]
                t1 = tp[0:128, 128:128 + P]
                nc.tensor.transpose(t0, hnew[0:P, 0:128], eye[0:P, 0:P])
                nc.tensor.transpose(t1, hnew[0:P, 128:256], eye[0:P, 0:P])
                if nch >= 2:
                    half = nch // 2
                    if pk < half:
                        d0, d1, col = par.sA0, par.sA1, 128 * pk
                    else:
                        d0, d1, col = par.sB0, par.sB1, 128 * (pk - half)
                    nc.vector.tensor_copy(d0[:, col:col + 128], t0)
                    nc.vector.tensor_copy(d1[:, col:col + 128], t1)
                else:
                    mp = P // 2
                    nc.vector.tensor_copy(par.sA0[:, 0:mp], t0[:, 0:mp])
                    nc.vector.tensor_copy(par.sB0[:, 0:mp], t0[:, mp:P])
                    nc.vector.tensor_copy(par.sA1[:, 0:mp], t1[:, 0:mp])
                    nc.vector.tensor_copy(par.sB1[:, 0:mp], t1[:, mp:P])

            def cin_aps(L, pk, P):
                if L == 13:
                    base = CIN13_OFF + 512 * pk
                    return (b16[0:P, base:base + 256],
                            b16[0:P, base + 256:base + 512])
                child = L + 1
                m_child = 2 * {l: m for (l, m, _) in PLAN}[L]
                nch_c = max(1, m_child // 128)
                cc = c_lev[child]
                if nch_c >= 2:
                    hf = nch_c // 2
                    return (cc[0:P, 512 * pk:512 * pk + 256],
                            cc[0:P, 512 * (hf + pk):512 * (hf + pk) + 256])
                if L == 9:
                    return (c_lev[10][0:64, 0:256], cB9[0:64, 0:256])
                return (c_lev[9][0:32, 0:256], cB8[0:32, 0:256])

            # ---------- fused levels 13..11 (software-pipelined) ----------
            for (L, M, row_off) in PLAN[:3]:
                nch = M // 128
                pend = []

                def emit_C(st):
                    (Lc, pkc, nchc, sgc, roff) = st
                    cA, cB = cin_aps(Lc, pkc, 128)
                    hnew = h_pool.tile([128, 512], F16)
                    cdst = c_lev[Lc][:, 512 * pkc:512 * pkc + 512]
                    cell_fused(128, sgc, cA, cB, cdst, hnew)
                    nc.sync.dma_start(
                        out_d[roff + 128 * pkc:roff + 128 * (pkc + 1), :],
                        hnew[0:128])
                    feed_pe(Lc, pkc, nchc, 128, hnew)

                for pk in range(nch):
                    if len(pend) == 2:
                        emit_C(pend.pop(0))
                    gc = gc_pool.tile([128, 1024], F32, tag="gc")
                    gd = gd_pool.tile([128, 1024], F32, tag="gd")
                    emit_A(L, pk, 128, gc, gd)
                    sg = sig_pool.tile([128, 2048], F16)
                    nc.scalar.activation(sg[0:128, 0:1024], gc[0:128],
                                         AF.Sigmoid)
                    nc.scalar.activation(sg[0:128, 1024:2048], gd[0:128],
                                         AF.Sigmoid)
                    pend.append((L, pk, nch, sg, row_off))
                while pend:
                    emit_C(pend.pop(0))

            # ---------- split levels 10, 9 and final level 8 ----------
            for (L, M, row_off) in PLAN[3:]:
                P = M
                gc = gc_pool.tile([128, 1024], F32, tag="gc")
                gd = gd_pool.tile([128, 1024], F32, tag="gd")
                emit_A(L, 0, P, gc, gd)
                sg = sig_pool.tile([128, 2048], F16)
                nc.scalar.activation(sg[0:P, 0:1024], gc[0:P],
                                     AF.Sigmoid)
                nc.scalar.activation(sg[0:P, 1024:2048], gd[0:P],
                                     AF.Sigmoid)
                cA, cB = cin_aps(L, 0, P)
                hnew = h_pool.tile([128, 512], F16)
                cdst = c_lev[L]
                cell_half(P, sg, 0, cA, cdst[0:P, 0:256], hnew[0:P, 0:256],
                          geng=False)
                if L > 8:
                    feed_pe(L, 0, 1, P, hnew)
                    if L == 10:
                        nc.sync.dma_start(cB9[0:64, 0:256],
                                          cdst[64:128, 0:256])
                    else:
                        nc.sync.dma_start(cB8[0:32, 0:256],
                                          cdst[32:64, 0:256])
                cell_half(P, sg, 1024, cB, cdst[0:P, 256:512],
                          hnew[0:P, 256:512])
                nc.sync.dma_start(out_d[row_off:row_off + P, :], hnew[0:P])
                if L == 8:
                    nc.sync.dma_start(out_d[2016:2048, :], cdst[0:32, 0:512])

    nc.compile()


def _get_built():
    global _BUILT
    if _BUILT is None:
        nc = bacc.Bacc("TRN2", target_bir_lowering=False, debug=False,
                       num_devices=N_CORES)
        _build_program(nc)
        _BUILT = nc
    return _BUILT


def kernel(types, a_idx, b_idx, emb, W_ih, W_hh, b_ih, b_hh):
    types = np.asarray(types, np.int32)
    emb = np.asarray(emb, np.float32)
    W_ih = np.asarray(W_ih, np.float32)
    W_hh = np.asarray(W_hh, np.float32)
    b = np.asarray(b_ih, np.float32) + np.asarray(b_hh, np.float32)

    XT = (W_ih @ emb.T + b[:, None]).astype(np.float32)      # [2048, 32]
    c_leaf = _sigmoid(XT[0:512]) * np.tanh(XT[1024:1536])
    h_leaf = _sigmoid(XT[1536:2048]) * np.tanh(c_leaf)
    M_A = W_hh[:, 0:256] @ h_leaf[0:256]
    M_B = W_hh[:, 256:512] @ h_leaf[0:256]
    cl256 = np.ascontiguousarray(c_leaf[0:256].T).astype(np.float16)

    w13p = np.vstack([M_A.T, M_B.T, XT.T])[:, GATE_PERM].copy()
    w13p[:, GCOLS] *= 2.0
    w13p = w13p.astype(np.float16)
    W_augT = np.vstack([W_hh.T, XT.T])[:, GATE_PERM].copy()
    W_augT[:, GCOLS] *= 2.0
    W_augT16 = W_augT.astype(np.float16)

    base16 = np.zeros((128, C16), np.float16)
    for k in range(4):
        base16[:, 2048 * k:2048 * (k + 1)] = W_augT16[128 * k:128 * (k + 1)]
    base16[:, WOH_OFF:WOH_OFF + 2048] = np.vstack([W_augT16[512:544]] * 4)
    base16[0:96, W13_OFF:W13_OFF + 2048] = w13p
    base16[:, EYE_OFF:EYE_OFF + 128] = np.eye(128, dtype=np.float16)

    sig = _perms()
    in_maps = []
    for j in range(N_CORES):
        n13 = (1 << 13) - 1 + 1024 * j + sig[13]
        la, lb = 2 * n13 + 1, 2 * n13 + 2
        oh3 = np.zeros((96, 1024), np.float16)
        m = np.arange(1024)
        oh3[types[la], m] = 1.0
        oh3[32 + types[lb], m] = 1.0
        oh3[64 + types[n13], m] = 1.0
        cin13 = np.concatenate([cl256[types[la]], cl256[types[lb]]],
                               axis=1)                       # [1024, 512] f16
        cimg = cin13.reshape(8, 128, 512).transpose(1, 0, 2).reshape(128, 4096)
        ohs = np.zeros((32, OHS_W), np.float16)
        for (L, M, _) in PLAN[1:]:
            nodes = (1 << L) - 1 + M * j + sig[L]
            ohs[types[nodes], OHS_OFF[L] + np.arange(M)] = 1.0

        b16 = base16.copy()
        b16[0:96, OH3_OFF:OH3_OFF + 1024] = oh3
        b16[:, OHS_COFF:OHS_COFF + OHS_W] = np.vstack([ohs] * 4)
        b16[:, CIN13_OFF:CIN13_OFF + 4096] = cimg
        in_maps.append({"big16": b16})

    nc = _get_built()
    res = run_bass_kernel_spmd(nc, in_maps, core_ids=list(range(N_CORES)))
    global LAST_RESULT
    LAST_RESULT = res

    out = np.empty((N, H2), np.float32)
    out[LEAF0:] = h_leaf.T[types[LEAF0:]]
    Hn = np.zeros((511, H2), np.float32)
    Cn = np.zeros((511, H2), np.float32)
    for j in range(N_CORES):
        r = res.results[j]["out"].astype(np.float32)
        for (L, M, off) in PLAN:
            base = (1 << L) - 1 + M * j
            out[base + sig[L]] = r[off:off + M]
        Hn[255 + 32 * j:255 + 32 * (j + 1)] = r[1984:2016]
        Cn[255 + 32 * j:255 + 32 * (j + 1)] = r[2016:2048]

    for L in range(7, -1, -1):
        ids = np.arange((1 << L) - 1, (1 << (L + 1)) - 1)
        a, bb = 2 * ids + 1, 2 * ids + 2
        hin = np.concatenate([Hn[a][:, 0:256], Hn[bb][:, 0:256]], axis=1)
        cin = np.concatenate([Cn[a][:, 0:256], Cn[bb][:, 0:256]], axis=1)
        gates = XT[:, types[ids]].T + hin @ W_hh.T
        ig, fg, gg, og = np.split(gates, 4, axis=1)
        c_new = _sigmoid(fg) * cin + _sigmoid(ig) * np.tanh(gg)
        h_new = _sigmoid(og) * np.tanh(c_new)
        Hn[ids] = h_new
        Cn[ids] = c_new
        out[ids] = h_new
    return out


# revision 18
# speedup vs baseline: 1.6513x; 1.0547x over previous
"""Binary tree-LSTM (BinaryTokenTreeModel) Trainium2 kernel, v2b.

Complete binary tree, depth 15 (N=32767), tree-LSTM state 2H=512,
gates 4*2H=2048, vocab 32.  Children feed parents the first H=256 dims
of (h, c).

Design (8 NeuronCores, data-parallel over the 8 level-3 subtrees):
  * Device computes levels 13..8 (2016 nodes/core); leaves are a host
    32-entry table; levels 7..0 (255 nodes) finish on host with
    level-batched GEMMs.
  * Unzip storage order (left children first): parent feeds are whole
    [128,128] transposes and contiguous column splits.
  * sigmoid-only activations (g-gate weight rows pre-scaled x2 so
    tanh(x) = 2*sigmoid(2x)-1): one 2048-wide ACT call per chunk plus
    one 512-wide for tanh(c) -- the scalar engine is the floor.
  * Gates accumulate in a single [128,2048] PSUM tile (4 banks, 2
    bufs); one-hot type rows (K=32) replicated 4x so the four quadrant
    matmuls run concurrently in distinct PE row groups.
  * All cell math in fp16 on the vector engine (2x DVE rate); c state,
    sigma tiles, and the output tensor are fp16 (rel err ~4e-3, budget
    2e-2).
  * Feeds: fused levels use HWDGE transpose-DMAs (no PE, no PSUM);
    split tail levels use PE transposes into the dead gates tile.
  * Software-pipelined emission (matmuls run two chunks ahead of the
    cell chain) keeps the PE dense so the HAM clock gate stays warm;
    junk matmuls cover the initial DMA window.

Self-contained: hardcodes all shapes; needs only numpy + the concourse
(bass) toolchain shipped with the environment.
"""

import sys

for _p in ("/opt/trn_rl_repo", "/root/.axon_site/_ro/trn_rl_repo"):
    if _p not in sys.path:
        sys.path.append(_p)

import numpy as np

import concourse.bacc as bacc
import concourse.mybir as mybir
import concourse.tile as tile
from concourse.alu_op_type import AluOpType
from concourse.bass_utils import run_bass_kernel_spmd

F32 = mybir.dt.float32
F16 = mybir.dt.float16
AF = mybir.ActivationFunctionType

N_CORES = 8
N = 32767
H = 256
H2 = 512
G = 2048
V = 32
LEAF0 = (1 << 14) - 1

# Permuted gate layout: [i_c f_c o_c g_c | i_d f_d o_d g_d]
GATE_PERM = np.concatenate([
    np.arange(0, 256), np.arange(512, 768),
    np.arange(1536, 1792), np.arange(1024, 1280),
    np.arange(256, 512), np.arange(768, 1024),
    np.arange(1792, 2048), np.arange(1280, 1536),
])
GCOLS = np.concatenate([np.arange(768, 1024), np.arange(1792, 2048)])

PLAN = [(13, 1024, 0), (12, 512, 1024), (11, 256, 1536), (10, 128, 1792),
        (9, 64, 1920), (8, 32, 1984)]
OHS_OFF = {12: 0, 11: 512, 10: 768, 9: 896, 8: 960}
OHS_W = 992
OUT_ROWS = 2048  # 2016 h rows + 32 level-8 c rows

# big16 column offsets (single fp16 input tensor)
WK_OFF = 0
WOH_OFF = 8192
W13_OFF = 10240
OH3_OFF = 12288
OHS_COFF = 13312
CIN13_OFF = 14304
EYE_OFF = 18400
C16 = 18528

_BUILT = None


def _sigmoid(x):
    return 1.0 / (1.0 + np.exp(-x))


def _perms():
    sig = {8: np.arange(32)}
    for L in range(9, 14):
        p = sig[L - 1]
        sig[L] = np.concatenate([2 * p, 2 * p + 1])
    return sig


class _Stor:
    def __init__(self, nc, L, M):
        self.M = M
        mk = lambda n: nc.alloc_sbuf_tensor(f"{n}_{L}", [128, M], F16).ap()
        self.sA0 = mk("sA0")
        self.sA1 = mk("sA1")
        self.sB0 = mk("sB0")
        self.sB1 = mk("sB1")


def _build_program(nc):
    big16 = nc.dram_tensor("big16", [128, C16], F16, kind="ExternalInput").ap()
    out_d = nc.dram_tensor("out", [OUT_ROWS, 512], F16, kind="ExternalOutput").ap()

    b16 = nc.alloc_sbuf_tensor("b16s", [128, C16], F16).ap()
    wk = [b16[:, WK_OFF + 2048 * k:WK_OFF + 2048 * (k + 1)] for k in range(4)]
    wohrep = b16[:, WOH_OFF:WOH_OFF + 2048]
    w13 = b16[0:96, W13_OFF:W13_OFF + 2048]
    oh3 = b16[0:96, OH3_OFF:OH3_OFF + 1024]
    ohsrep = b16[:, OHS_COFF:OHS_COFF + OHS_W]
    cin13 = b16[:, CIN13_OFF:CIN13_OFF + 4096]
    eye = b16[:, EYE_OFF:EYE_OFF + 128]

    c_lev = {L: nc.alloc_sbuf_tensor(f"c{L}", [128, (max(M, 128) // 128) * 512],
                                     F16).ap()
             for (L, M, _) in PLAN}
    stor = {L: _Stor(nc, L, M) for (L, M, _) in PLAN if L != 13}
    cB9 = nc.alloc_sbuf_tensor("cB9", [64, 256], F16).ap()
    cB8 = nc.alloc_sbuf_tensor("cB8", [32, 256], F16).ap()

    with tile.TileContext(nc) as tc:
        import contextlib

        with contextlib.ExitStack() as ctx:
            gc_pool = ctx.enter_context(
                tc.tile_pool(name="gc", bufs=2, space="PSUM"))
            gd_pool = ctx.enter_context(
                tc.tile_pool(name="gd", bufs=1, space="PSUM"))
            tp_pool = ctx.enter_context(
                tc.tile_pool(name="tp", bufs=1, space="PSUM"))
            junk_pool = ctx.enter_context(
                tc.tile_pool(name="junk", bufs=1, space="PSUM"))
            sig_pool = ctx.enter_context(tc.tile_pool(name="sig", bufs=4))
            s2c_pool = ctx.enter_context(tc.tile_pool(name="s2c", bufs=3))
            work_pool = ctx.enter_context(tc.tile_pool(name="wrk", bufs=12))
            h_pool = ctx.enter_context(tc.tile_pool(name="hh", bufs=4))

            # ---- input DMA, ordered by first use, 2 HWDGE queues ----
            nc.scalar.dma_start(b16[:, W13_OFF:W13_OFF + 2048],
                                big16[:, W13_OFF:W13_OFF + 2048])  # w13
            nc.sync.dma_start(b16[:, OH3_OFF:OH3_OFF + 1024],
                              big16[:, OH3_OFF:OH3_OFF + 1024])    # oh3
            nc.scalar.dma_start(b16[:, CIN13_OFF:C16],
                              big16[:, CIN13_OFF:C16])             # cin13+eye
            nc.sync.dma_start(b16[:, OHS_COFF:OHS_COFF + OHS_W],
                              big16[:, OHS_COFF:OHS_COFF + OHS_W])
            nc.scalar.dma_start(b16[:, 0:4096], big16[:, 0:4096])  # wk01
            nc.sync.dma_start(b16[:, 4096:8192], big16[:, 4096:8192])
            nc.scalar.dma_start(b16[:, 8192:10240], big16[:, 8192:10240])

            # ---- HAM warm-up ----
            wtile = junk_pool.tile([128, 512], F32, tag="junk")
            for _ in range(24):
                nc.tensor.matmul(wtile[0:128, 0:512],
                                 b16[:, W13_OFF:W13_OFF + 128],
                                 b16[:, W13_OFF:W13_OFF + 512],
                                 start=True, stop=True, skip_group_check=True)

            def emit_A(L, pk, P, gc, gd):
                c0 = pk * 128
                quads = [gc[0:P, 0:512], gc[0:P, 512:1024],
                         gd[0:P, 0:512], gd[0:P, 512:1024]]
                if L == 13:
                    lhs = oh3[:, c0:c0 + P]
                    for b in range(4):
                        nc.tensor.matmul(quads[b], lhs,
                                         w13[:, 512 * b:512 * (b + 1)],
                                         start=True, stop=True,
                                         skip_group_check=True)
                else:
                    st = stor[L]
                    lhs4 = [st.sA0[:, c0:c0 + P], st.sA1[:, c0:c0 + P],
                            st.sB0[:, c0:c0 + P], st.sB1[:, c0:c0 + P]]
                    for b in range(4):
                        for k in range(4):
                            nc.tensor.matmul(
                                quads[b], lhs4[k],
                                wk[k][:, 512 * b:512 * (b + 1)],
                                start=(k == 0), stop=False,
                                skip_group_check=True)
                    off = OHS_OFF[L]
                    for b in range(4):
                        nc.tensor.matmul(
                            quads[b],
                            ohsrep[32 * b:32 * b + 32, off + c0:off + c0 + P],
                            wohrep[32 * b:32 * b + 32, 512 * b:512 * (b + 1)],
                            start=False, stop=True, skip_group_check=True,
                            tile_position=(32 * b, 0))

            def cell_half(P, sg, coff, cin_ap, cdst, hdst, geng=True):
                """One 256-dim half: c = σf·c_in + σi·(2σg−1);
                h = σo·(2σ(2c)−1).  All fp16, 2D contiguous APs."""
                si = sg[0:P, coff + 0:coff + 256]
                sf = sg[0:P, coff + 256:coff + 512]
                so = sg[0:P, coff + 512:coff + 768]
                sgg = sg[0:P, coff + 768:coff + 1024]
                tg = work_pool.tile([128, 256], F16)
                nc.vector.tensor_scalar(tg[0:P], sgg, 2.0, -1.0,
                                        AluOpType.mult, AluOpType.add)
                p2 = work_pool.tile([128, 256], F16)
                nc.vector.tensor_mul(p2[0:P], si, tg[0:P])
                fc = work_pool.tile([128, 256], F16)
                (nc.gpsimd if geng else nc.vector).tensor_mul(
                    fc[0:P], sf, cin_ap)
                nc.vector.tensor_add(cdst, fc[0:P], p2[0:P])
                s2c = s2c_pool.tile([128, 256], F16)
                nc.scalar.activation(s2c[0:P], cdst, AF.Sigmoid, scale=2.0)
                tc_ = work_pool.tile([128, 256], F16)
                nc.vector.tensor_scalar(tc_[0:P], s2c[0:P], 2.0, -1.0,
                                        AluOpType.mult, AluOpType.add)
                (nc.gpsimd if geng else nc.vector).tensor_mul(
                    hdst, so, tc_[0:P])

            def cell_fused(P, sg, cA, cB, cdst, hnew):
                """Both halves; fc on gpsimd, single [P,512] s2c call."""
                for half, coff in ((0, 0), (1, 1024)):
                    si = sg[0:P, coff + 0:coff + 256]
                    sf = sg[0:P, coff + 256:coff + 512]
                    sgg = sg[0:P, coff + 768:coff + 1024]
                    cin_ap = cA if half == 0 else cB
                    cs = slice(256 * half, 256 * half + 256)
                    tg = work_pool.tile([128, 256], F16)
                    nc.vector.tensor_scalar(tg[0:P], sgg, 2.0, -1.0,
                                            AluOpType.mult, AluOpType.add)
                    p2 = work_pool.tile([128, 256], F16)
                    nc.vector.tensor_mul(p2[0:P], si, tg[0:P])
                    fc = work_pool.tile([128, 256], F16)
                    nc.gpsimd.tensor_mul(fc[0:P], sf, cin_ap)
                    nc.vector.tensor_add(cdst[0:P, cs], fc[0:P], p2[0:P])
                s2c = s2c_pool.tile([128, 512], F16)
                nc.scalar.activation(s2c[0:P], cdst[0:P, 0:512],
                                     AF.Sigmoid, scale=2.0)
                for half, coff in ((0, 0), (1, 1024)):
                    so = sg[0:P, coff + 512:coff + 768]
                    cs = slice(256 * half, 256 * half + 256)
                    tc_ = work_pool.tile([128, 256], F16)
                    nc.vector.tensor_scalar(tc_[0:P], s2c[0:P, cs], 2.0, -1.0,
                                            AluOpType.mult, AluOpType.add)
                    (nc.gpsimd if half else nc.vector).tensor_mul(
                        hnew[0:P, cs], sg[0:P, coff + 512:coff + 768],
                        tc_[0:P])

            def feed_pe(L, pk, nch, P, hnew):
                """Feed: fp16 PE transposes of h crit into the transpose
                scratch, then column copies into the parent stationaries."""
                par = stor[L - 1]
                tp = tp_pool.tile([128, 256], F16)
                t0 = tp[0:128, 0:P]
                t1 = tp[0:128, 128:128 + P]
                nc.tensor.transpose(t0, hnew[0:P, 0:128], eye[0:P, 0:P])
                nc.tensor.transpose(t1, hnew[0:P, 128:256], eye[0:P, 0:P])
                if nch >= 2:
                    half = nch // 2
                    if pk < half:
                        d0, d1, col = par.sA0, par.sA1, 128 * pk
                    else:
                        d0, d1, col = par.sB0, par.sB1, 128 * (pk - half)
                    nc.vector.tensor_copy(d0[:, col:col + 128], t0)
                    nc.vector.tensor_copy(d1[:, col:col + 128], t1)
                else:
                    mp = P // 2
                    nc.vector.tensor_copy(par.sA0[:, 0:mp], t0[:, 0:mp])
                    nc.vector.tensor_copy(par.sB0[:, 0:mp], t0[:, mp:P])
                    nc.vector.tensor_copy(par.sA1[:, 0:mp], t1[:, 0:mp])
                    nc.vector.tensor_copy(par.sB1[:, 0:mp], t1[:, mp:P])

            def cin_aps(L, pk, P):
                if L == 13:
                    base = CIN13_OFF + 512 * pk
                    return (b16[0:P, base:base + 256],
                            b16[0:P, base + 256:base + 512])
                child = L + 1
                m_child = 2 * {l: m for (l, m, _) in PLAN}[L]
                nch_c = max(1, m_child // 128)
                cc = c_lev[child]
                if nch_c >= 2:
                    hf = nch_c // 2
                    return (cc[0:P, 512 * pk:512 * pk + 256],
                            cc[0:P, 512 * (hf + pk):512 * (hf + pk) + 256])
                if L == 9:
                    return (c_lev[10][0:64, 0:256], cB9[0:64, 0:256])
                return (c_lev[9][0:32, 0:256], cB8[0:32, 0:256])

            # ---------- fused levels 13..11 (software-pipelined) ----------
            for (L, M, row_off) in PLAN[:3]:
                nch = M // 128
                pend = []

                def emit_C(st):
                    (Lc, pkc, nchc, gcd, sgc, roff) = st
                    if Lc == 13:
                        jt = junk_pool.tile([128, 512], F32, tag="junk")
                        for _ in range(2):
                            nc.tensor.matmul(jt[0:128, 0:512],
                                             b16[:, W13_OFF:W13_OFF + 128],
                                             b16[:, W13_OFF:W13_OFF + 512],
                                             start=True, stop=True,
                                             skip_group_check=True)
                    cA, cB = cin_aps(Lc, pkc, 128)
                    hnew = h_pool.tile([128, 512], F16)
                    cdst = c_lev[Lc][:, 512 * pkc:512 * pkc + 512]
                    cell_fused(128, sgc, cA, cB, cdst, hnew)
                    nc.sync.dma_start(
                        out_d[roff + 128 * pkc:roff + 128 * (pkc + 1), :],
                        hnew[0:128])
                    feed_pe(Lc, pkc, nchc, 128, hnew)

                for pk in range(nch):
                    if len(pend) == 2:
                        emit_C(pend.pop(0))
                    gc = gc_pool.tile([128, 1024], F32, tag="gc")
                    gd = gd_pool.tile([128, 1024], F32, tag="gd")
                    emit_A(L, pk, 128, gc, gd)
                    sg = sig_pool.tile([128, 2048], F16)
                    nc.scalar.activation(sg[0:128, 1024:2048], gd[0:128],
                                         AF.Sigmoid)
                    nc.scalar.activation(sg[0:128, 0:1024], gc[0:128],
                                         AF.Sigmoid)
                    pend.append((L, pk, nch, gc, sg, row_off))
                while pend:
                    emit_C(pend.pop(0))

            # ---------- split levels 10, 9 and final level 8 ----------
            for (L, M, row_off) in PLAN[3:]:
                P = M
                gc = gc_pool.tile([128, 1024], F32, tag="gc")
                gd = gd_pool.tile([128, 1024], F32, tag="gd")
                emit_A(L, 0, P, gc, gd)
                sg = sig_pool.tile([128, 2048], F16)
                nc.scalar.activation(sg[0:P, 0:1024], gc[0:P],
                                     AF.Sigmoid)
                nc.scalar.activation(sg[0:P, 1024:2048], gd[0:P],
                                     AF.Sigmoid)
                cA, cB = cin_aps(L, 0, P)
                hnew = h_pool.tile([128, 512], F16)
                cdst = c_lev[L]
                cell_half(P, sg, 0, cA, cdst[0:P, 0:256], hnew[0:P, 0:256],
                          geng=False)
                if L > 8:
                    feed_pe(L, 0, 1, P, hnew)
                    jt = junk_pool.tile([128, 512], F32, tag="junk")
                    for _ in range(3):
                        nc.tensor.matmul(jt[0:128, 0:512],
                                         b16[:, W13_OFF:W13_OFF + 128],
                                         b16[:, W13_OFF:W13_OFF + 512],
                                         start=True, stop=True,
                                         skip_group_check=True)
                    if L == 10:
                        nc.sync.dma_start(cB9[0:64, 0:256],
                                          cdst[64:128, 0:256])
                    else:
                        nc.sync.dma_start(cB8[0:32, 0:256],
                                          cdst[32:64, 0:256])
                cell_half(P, sg, 1024, cB, cdst[0:P, 256:512],
                          hnew[0:P, 256:512])
                nc.sync.dma_start(out_d[row_off:row_off + P, :], hnew[0:P])
                if L == 8:
                    nc.sync.dma_start(out_d[2016:2048, :], cdst[0:32, 0:512])

    nc.compile()


def _get_built():
    global _BUILT
    if _BUILT is None:
        nc = bacc.Bacc("TRN2", target_bir_lowering=False, debug=False,
                       num_devices=N_CORES)
        _build_program(nc)
        _BUILT = nc
    return _BUILT


def kernel(types, a_idx, b_idx, emb, W_ih, W_hh, b_ih, b_hh):
    types = np.asarray(types, np.int32)
    emb = np.asarray(emb, np.float32)
    W_ih = np.asarray(W_ih, np.float32)
    W_hh = np.asarray(W_hh, np.float32)
    b = np.asarray(b_ih, np.float32) + np.asarray(b_hh, np.float32)

    XT = (W_ih @ emb.T + b[:, None]).astype(np.float32)      # [2048, 32]
    c_leaf = _sigmoid(XT[0:512]) * np.tanh(XT[1024:1536])
    h_leaf = _sigmoid(XT[1536:2048]) * np.tanh(c_leaf)
    M_A = W_hh[:, 0:256] @ h_leaf[0:256]
    M_B = W_hh[:, 256:512] @ h_leaf[0:256]
    cl256 = np.ascontiguousarray(c_leaf[0:256].T).astype(np.float16)

    w13p = np.vstack([M_A.T, M_B.T, XT.T])[:, GATE_PERM].copy()
    w13p[:, GCOLS] *= 2.0
    w13p = w13p.astype(np.float16)
    W_augT = np.vstack([W_hh.T, XT.T])[:, GATE_PERM].copy()
    W_augT[:, GCOLS] *= 2.0
    W_augT16 = W_augT.astype(np.float16)

    base16 = np.zeros((128, C16), np.float16)
    for k in range(4):
        base16[:, 2048 * k:2048 * (k + 1)] = W_augT16[128 * k:128 * (k + 1)]
    base16[:, WOH_OFF:WOH_OFF + 2048] = np.vstack([W_augT16[512:544]] * 4)
    base16[0:96, W13_OFF:W13_OFF + 2048] = w13p
    base16[:, EYE_OFF:EYE_OFF + 128] = np.eye(128, dtype=np.float16)

    sig = _perms()
    in_maps = []
    for j in range(N_CORES):
        n13 = (1 << 13) - 1 + 1024 * j + sig[13]
        la, lb = 2 * n13 + 1, 2 * n13 + 2
        oh3 = np.zeros((96, 1024), np.float16)
        m = np.arange(1024)
        oh3[types[la], m] = 1.0
        oh3[32 + types[lb], m] = 1.0
        oh3[64 + types[n13], m] = 1.0
        cin13 = np.concatenate([cl256[types[la]], cl256[types[lb]]],
                               axis=1)                       # [1024, 512] f16
        cimg = cin13.reshape(8, 128, 512).transpose(1, 0, 2).reshape(128, 4096)
        ohs = np.zeros((32, OHS_W), np.float16)
        for (L, M, _) in PLAN[1:]:
            nodes = (1 << L) - 1 + M * j + sig[L]
            ohs[types[nodes], OHS_OFF[L] + np.arange(M)] = 1.0

        b16 = base16.copy()
        b16[0:96, OH3_OFF:OH3_OFF + 1024] = oh3
        b16[:, OHS_COFF:OHS_COFF + OHS_W] = np.vstack([ohs] * 4)
        b16[:, CIN13_OFF:CIN13_OFF + 4096] = cimg
        in_maps.append({"big16": b16})

    nc = _get_built()
    res = run_bass_kernel_spmd(nc, in_maps, core_ids=list(range(N_CORES)))
    global LAST_RESULT
    LAST_RESULT = res

    out = np.empty((N, H2), np.float32)
    out[LEAF0:] = h_leaf.T[types[LEAF0:]]
    Hn = np.zeros((511, H2), np.float32)
    Cn = np.zeros((511, H2), np.float32)
    for j in range(N_CORES):
        r = res.results[j]["out"].astype(np.float32)
        for (L, M, off) in PLAN:
            base = (1 << L) - 1 + M * j
            out[base + sig[L]] = r[off:off + M]
        Hn[255 + 32 * j:255 + 32 * (j + 1)] = r[1984:2016]
        Cn[255 + 32 * j:255 + 32 * (j + 1)] = r[2016:2048]

    for L in range(7, -1, -1):
        ids = np.arange((1 << L) - 1, (1 << (L + 1)) - 1)
        a, bb = 2 * ids + 1, 2 * ids + 2
        hin = np.concatenate([Hn[a][:, 0:256], Hn[bb][:, 0:256]], axis=1)
        cin = np.concatenate([Cn[a][:, 0:256], Cn[bb][:, 0:256]], axis=1)
        gates = XT[:, types[ids]].T + hin @ W_hh.T
        ig, fg, gg, og = np.split(gates, 4, axis=1)
        c_new = _sigmoid(fg) * cin + _sigmoid(ig) * np.tanh(gg)
        h_new = _sigmoid(og) * np.tanh(c_new)
        Hn[ids] = h_new
        Cn[ids] = c_new
        out[ids] = h_new
    return out
